# revision 37
# baseline (speedup 1.0000x reference)
"""Trainium2 Bass kernel for nn_CycleGNN (8-step projected-direction solver).

Contract: kernel(**inputs) takes the FULL unsharded numpy inputs (keyed as in
setup_inputs()) and returns the full output (preds, labels), each
[131072, 8] float32.  Internally shards the 64 graphs across 8 NeuronCores
(8 graphs per core, graphs never interact -> no collectives), runs a Tile
kernel via run_bass_kernel_spmd, and re-assembles on the host.

Device-side design (per core, 8 graphs, 16384 nodes):
 - The 8 graphs are split into two HALVES (graphs 0-3 / 4-7) with fully
   independent per-half tiles; the two halves' instruction streams are
   interleaved so each half's serial DVE/DMA chains hide under the other
   half's PE bursts (mlp -> d-chain -> einsum1 -> einsum2 -> line search
   is one serial chain per half, but the halves share no data).
 - per-node state is "p-major banded" [64, 128] per half:
   graph q owns partitions [16q, 16q+16); node-within-graph (p%16)*128+c.
 - BOTH P and P^T are SBUF-resident in fp8-e3m4 (scaled by 32), so the
   steady-state loop does no HBM traffic for the projection.
 - einsum1 (df = P^T d) and einsum2 (y = P df) run as 4-way column-tiled
   matvecs (4 graphs concurrent in the PE array via tile_position).
 - line search uses the max-ratio form r = y * (1/xs): the per-element
   divide is replaced by one ACT-LUT reciprocal of xs per step, and the
   per-graph min + broadcast runs on tiny gather/broadcast DMAs (no PE).
"""

import numpy as np
import ml_dtypes

import bass_rust
import concourse.bass as bass
import concourse.tile as tile
from concourse import mybir
from concourse.bass_utils import run_bass_kernel_spmd
from concourse.masks import make_identity

F32 = mybir.dt.float32
BF16 = mybir.dt.bfloat16
FP8 = mybir.dt.float8e3
BF = ml_dtypes.bfloat16
F8 = ml_dtypes.float8_e3m4
PSCALE = 32.0    # P and P^T stored as fp8e3 * PSCALE (absmax ~3.8 < 15.5)

B = 64          # graphs
NMAX = 2048     # nodes per graph (equal-size, sorted vals_batch)
F = 512         # projection basis dim
HID = 128
NFEAT = 64
NUM_STEPS = 8
STEP_ALPHA = 5.0
NCORES = 8
GPC = B // NCORES            # graphs per core = 8
NPC = GPC * NMAX             # nodes per core = 16384
NCH = NMAX // 128            # n-chunks per graph = 16
FCH = F // 128               # f-chunks = 4
GPH = GPC // 2               # graphs per half = 4
NPH = GPH * NMAX             # nodes per half = 8192

AX = mybir.AxisListType
OP = mybir.AluOpType
ACT = mybir.ActivationFunctionType

_COMPILED = {}


def _split_sync_waits(nc, maxw=1):
    """Walrus in this container accepts at most one sync wait per
    instruction; split extra waits into preceding engine-local NoOps."""
    ctr = 0
    for f in nc.m.functions:
        for bb in f.blocks:
            insts = bb.instructions
            out = []
            changed = False
            for ins in insts:
                si = ins.sync_info
                waits = list(si.on_wait) if si is not None else []
                if len(waits) > maxw:
                    reg_waits = [w for w in waits if w.wait_reg is not None]
                    imm_waits = [w for w in waits if w.wait_reg is None]
                    nkeep = max(0, maxw - len(reg_waits))
                    keep = imm_waits[:nkeep]
                    extra = imm_waits[nkeep:]
                    for i in range(0, len(extra), maxw):
                        ctr += 1
                        nop = mybir.InstNoOp(name=f"wsplit-{ctr}", ins=[], outs=[])
                        nop.engine = ins.engine
                        nop.sync_info = bass_rust.SyncInfo(
                            on_wait=extra[i : i + maxw], on_update=[]
                        )
                        out.append(nop)
                    ins.sync_info = bass_rust.SyncInfo(
                        on_wait=reg_waits + keep, on_update=list(si.on_update)
                    )
                    changed = True
                out.append(ins)
            if changed:
                bb.instructions = out
    return ctr


def _tau_schedule():
    taus = []
    tau = 0.01
    for _ in range(NUM_STEPS):
        taus.append(tau)
        tau = max(tau * 0.5, 1e-5)
    return taus


def build_nc(num_steps=NUM_STEPS, debug=False):
    nc = bass.Bass()

    # ---------------- I/O ----------------
    P_d = nc.declare_dram_parameter("P", [128, GPC, NCH, F], FP8, isOutput=False)
    PT_d = nc.declare_dram_parameter("PT", [128, GPC, FCH, NMAX], FP8, isOutput=False)
    nfT_d = nc.declare_dram_parameter("nfT", [NFEAT, NPC], BF16, isOutput=False)
    xs0_d = nc.declare_dram_parameter("xs0", [128, 128], F32, isOutput=False)
    xsol_d = nc.declare_dram_parameter("xsol", [128, 128], F32, isOutput=False)
    w1_d = nc.declare_dram_parameter("w1", [NFEAT + 1, HID], BF16, isOutput=False)
    b1_d = nc.declare_dram_parameter("b1", [HID, 1], F32, isOutput=False)
    w2_d = nc.declare_dram_parameter("w2", [HID, 1], BF16, isOutput=False)
    b2_d = nc.declare_dram_parameter("b2", [1, 1], F32, isOutput=False)
    seg_d = nc.declare_dram_parameter("seg", [64, 64], F32, isOutput=False)
    seg8_d = nc.declare_dram_parameter("seg8", [4, 64], F32, isOutput=False)

    preds_o = nc.declare_dram_parameter("preds", [NUM_STEPS, NPC], F32, isOutput=True)
    # xs snapshot at the START of each step; labels are computed on the host
    xs_o = nc.declare_dram_parameter("xs_o", [NUM_STEPS, 128, 128], F32, isOutput=True)

    taus = _tau_schedule()

    with tile.TileContext(nc) as tc:
        with (
            tc.tile_pool(name="res", bufs=1) as res,            # resident singles
            tc.tile_pool(name="hp", bufs=8) as hp,              # relu'd hidden chunks
            tc.tile_pool(name="st", bufs=1) as st,              # per-half state
            tc.tile_pool(name="sm", bufs=1) as sm,              # small temps
            tc.tile_pool(name="mh_ps", bufs=4, space="PSUM") as mh_ps,
            tc.tile_pool(name="mi_ps", bufs=1, space="PSUM") as mi_ps,
            tc.tile_pool(name="e_ps", bufs=3, space="PSUM") as e_ps,
        ):
            # ---------------- constants / residents ----------------
            identb = res.tile([128, 128], BF16, tag="identb")
            make_identity(nc, identb)

            identf = res.tile([64, 64], F32, tag="identf")
            make_identity(nc, identf)
            seg = res.tile([64, 64], F32, tag="seg")
            nc.sync.dma_start(out=seg, in_=seg_d[:])
            seg8 = res.tile([4, 64], F32, tag="seg8")
            nc.sync.dma_start(out=seg8, in_=seg8_d[:])

            w1 = res.tile([NFEAT + 1, HID], BF16, tag="w1")
            nc.sync.dma_start(out=w1, in_=w1_d[:])
            b1c = res.tile([HID, 1], F32, tag="b1c")
            nc.sync.dma_start(out=b1c, in_=b1_d[:])
            w2 = res.tile([HID, 1], BF16, tag="w2")
            nc.sync.dma_start(out=w2, in_=w2_d[:])
            b2c = res.tile([128, 1], F32, tag="b2c")
            nc.sync.dma_start(
                out=b2c,
                in_=bass.AP(tensor=b2_d, offset=0, ap=[[0, 128], [1, 1]]),
            )

            # per-half mlp moving operand: rows 0..63 node features, row 64 = xs
            rhsx = [res.tile([NFEAT + 1, NPH], BF16, tag=f"rhsx{h}", name="rhsx") for h in (0, 1)]
            for h in (0, 1):
                nc.scalar.dma_start(
                    out=rhsx[h][0:NFEAT, :], in_=nfT_d[:, NPH * h : NPH * (h + 1)]
                )

            # per-half state (all on partitions 0..63)
            xs = [st.tile([64, 128], F32, tag=f"xs{h}", name="xs") for h in (0, 1)]
            xs_inv = [st.tile([64, 128], F32, tag=f"xsi{h}", name="xsi") for h in (0, 1)]
            rterm = [st.tile([64, 128], F32, tag=f"rt{h}", name="rt") for h in (0, 1)]
            pred = [st.tile([64, 128], BF16, tag=f"pred{h}", name="pred") for h in (0, 1)]
            y_pm = [st.tile([64, 128], BF16, tag=f"y{h}", name="ypm") for h in (0, 1)]
            for h in (0, 1):
                nc.gpsimd.dma_start(out=xs[h], in_=xs0_d[64 * h : 64 * h + 64, :])

            # resident P and P^T (fp8 * 32).  Chunked so step-0's small DMAs
            # on the same queues don't wait ~12us behind a monolithic load;
            # PT chunks are emitted inside step 0 (see loop below).
            sbP = res.tile([128, GPC, NCH, F], FP8, tag="sbP")
            sbPT = res.tile([128, GPC, FCH, NMAX], FP8, tag="sbPT")
            # priority order per queue: the first graphs' P and PT come
            # before the later graphs' P (e2_0(0) needs PT g0-3 early)
            for g in range(4):
                eng = (nc.sync, nc.gpsimd)[g % 2]
                eng.dma_start(out=sbP[:, g], in_=P_d[:, g])
            for g in range(4):
                eng = (nc.sync, nc.gpsimd)[g % 2]
                eng.dma_start(out=sbPT[:, g], in_=PT_d[:, g])
            for g in range(4, GPC):
                eng = (nc.sync, nc.gpsimd)[g % 2]
                eng.dma_start(out=sbP[:, g], in_=P_d[:, g])

            def emit_pt_load(h, part):
                # alternate queues per graph so neither DMA queue is
                # blocked for the whole 4MB
                for g4 in (part,) if part is not None else range(GPH):
                    g = GPH * h + g4
                    eng = (nc.sync, nc.gpsimd)[g4 % 2]
                    eng.dma_start(out=sbPT[:, g], in_=PT_d[:, g])

            # ---- step-0 init per half: xs row into rhsx, xs_inv, rterm ----
            for h in (0, 1):
                xbf = sm.tile([64, 128], BF16, tag=f"xbf{h}", name="xbf")
                nc.vector.tensor_copy(xbf, xs[h])
                nc.scalar.dma_start(
                    out=rhsx[h][NFEAT : NFEAT + 1, :].rearrange(
                        "o (p c) -> o p c", p=64
                    ),
                    in_=xbf,
                )
                nc.vector.reciprocal(out=xs_inv[h], in_=xs[h])
                t0 = taus[0]
                rtt = sm.tile([64, 128], F32, tag=f"rtt{h}", name="rtt")
                nc.vector.tensor_scalar(
                    out=rtt, in0=xs[h], scalar1=float(1.0 / (3.0 * t0)),
                    scalar2=float(1.0 / 3.0), op0=OP.mult, op1=OP.add,
                )
                nc.vector.reciprocal(out=rterm[h], in_=rtt)
                nc.gpsimd.dma_start(
                    out=xs_o[0][64 * h : 64 * h + 64, :], in_=xs[h]
                )

            # =================== helper emitters ===================

            def emit_mlp(s, h, injects=None):
                """MLP for half h of step s.  PE order: hg0 hg1 o0 hg2 o1
                hg3 o2 o3 (out-round r consumes hidden group r = chunks
                {4q+r}).  injects: {point: [fn, ...]} emitted at sequence
                points 1=after hg0, 2=after hg1, 3=after o0, 4=after hg2,
                5=after o1, 6=after hg3, 7=after o2, 8=end."""
                injects = injects or {}

                def at(p):
                    for fn in injects.get(p, ()):
                        fn()

                prow = sm.tile([128, 2048], BF16, tag=f"prow{h}", name="prow")
                hq = {}
                tog = [0]

                def hgroup(r):
                    for q in range(4):
                        c = 4 * q + r
                        hps = mh_ps.tile([128, 512], F32, tag="mh", name="hps")
                        nc.tensor.matmul(
                            hps, w1, rhsx[h][:, 512 * c : 512 * (c + 1)],
                            start=True, stop=True,
                        )
                        hpos = hp.tile([128, 512], BF16, tag="h", name="hpos")
                        if tog[0] % 16 in (0, 2, 5, 7, 9, 11, 14):
                            nc.vector.tensor_scalar(
                                out=hpos, in0=hps, scalar1=b1c, scalar2=0.0,
                                op0=OP.add, op1=OP.max,
                            )
                        else:
                            nc.scalar.activation(
                                out=hpos, in_=hps, func=ACT.Relu, bias=b1c
                            )
                        tog[0] += 1
                        hq[c] = hpos

                def oround(r):
                    pp = mi_ps.tile([128, 512], F32, tag="mi", name="pp")
                    for q in range(4):
                        nc.tensor.matmul(
                            pp[32 * q : 32 * q + 1, :],
                            w2, hq[4 * q + r],
                            start=True, stop=True,
                            tile_position=(0, 32 * q),
                        )
                    nc.scalar.activation(
                        out=prow[:, 512 * r : 512 * (r + 1)], in_=pp,
                        func=ACT.Identity, bias=b2c,
                    )
                    # scatter this round's 512-node slice of each graph
                    # into the pred p-major band right away (on V/S queues
                    # for step 0, whose sync/gpsimd queues carry P/PT loads)
                    for q in range(4):
                        if s == 0:
                            eng = nc.scalar
                        elif r == 3:
                            eng = (nc.scalar, nc.sync, nc.gpsimd, nc.scalar)[q]
                        else:
                            eng = (nc.sync, nc.gpsimd)[(r + q) % 2]
                        eng.dma_start(
                            out=pred[h][16 * q + 4 * r : 16 * q + 4 * r + 4, :],
                            in_=prow[
                                32 * q : 32 * q + 1, 512 * r : 512 * (r + 1)
                            ].rearrange("o (p c) -> o p c", p=4),
                        )

                hgroup(0)
                at(1)
                hgroup(1)
                at(2)
                oround(0)
                at(3)
                hgroup(2)
                at(4)
                oround(1)
                at(5)
                hgroup(3)
                at(6)
                oround(2)
                at(7)
                oround(3)
                at(8)
                # preds output straight from row staging (flat node order)
                nc.gpsimd.dma_start(
                    out=preds_o[s, NPH * h : NPH * (h + 1)].rearrange(
                        "(q c) -> q c", q=4
                    ),
                    in_=prow.rearrange("(q o) c -> q o c", q=4)[:, 0:1, :],
                )
                at(9)

            def emit_dchain(h):
                """|pred|_1 per graph -> pscale; d_bf = pred*pscale + rterm.
                Emitted via two parts so the PE op (seg matmul) can sit at a
                chosen PE-queue slot."""
                pp_abs = sm.tile([64, 1], F32, tag=f"pabs{h}", name="pabs")
                junk = sm.tile([64, 128], F32, tag=f"junk{h}", name="junk")
                nc.scalar.activation(
                    out=junk, in_=pred[h], func=ACT.Abs, accum_out=pp_abs
                )
                gs = mi_ps.tile([64, 1], F32, tag="mi", name="gs")
                nc.tensor.matmul(gs, seg, pp_abs, start=True, stop=True)
                pscale = sm.tile([64, 1], F32, tag=f"psc{h}", name="psc")
                nc.vector.reciprocal(pscale, gs)
                d_bf = sm.tile([64, 128], BF16, tag=f"dbf{h}", name="dbf")
                nc.vector.scalar_tensor_tensor(
                    out=d_bf, in0=pred[h], scalar=pscale, in1=rterm[h],
                    op0=OP.mult, op1=OP.add,
                )
                return d_bf

            def emit_dT(h, d_bf):
                """d_bf [64,128] -> d_cols [128,64] via PE transpose.
                d_cols column 16*g4+k = d for (graph g4, node chunk k)."""
                dct = mh_ps.tile([128, 64], BF16, tag="mh", name="dct")
                nc.tensor.transpose(dct, d_bf, identb[0:64, 0:64])
                d_cols = sm.tile([128, 64], BF16, tag=f"dc{h}", name="dc")
                nc.vector.tensor_copy(d_cols, dct)
                return d_cols

            def emit_e1(h, d_cols, injects=None):
                """einsum1: dfp row 32*g4 = 32*df[g]  (4-way col-tiled).
                injects: {k: [fn, ...]} emitted after k-group k."""
                injects = injects or {}
                dfp = e_ps.tile([128, F], F32, tag="e", name="dfp")
                for k in range(NCH):
                    for g4 in range(4):
                        g = GPH * h + g4
                        nc.tensor.matmul(
                            dfp[32 * g4 : 32 * g4 + 1, :],
                            d_cols[:, 16 * g4 + k : 16 * g4 + k + 1],
                            sbP[:, g, k, :],
                            start=(k == 0),
                            stop=(k == NCH - 1),
                            tile_position=(0, 32 * g4),
                        )
                    for fn in injects.get(k, ()):
                        fn()
                return dfp

            def emit_df_evac(h, dfp):
                # split across V and S so the stage is ready ~0.35us sooner
                dfstage = sm.tile([128, F], BF16, tag=f"dfs{h}", name="dfs")
                nc.scalar.activation(
                    out=dfstage[:, 0:256], in_=dfp[:, 0:256],
                    func=ACT.Identity, scale=float(1.0 / PSCALE),
                )
                nc.vector.tensor_scalar(
                    out=dfstage[:, 256:512], in0=dfp[:, 256:512],
                    scalar1=float(1.0 / PSCALE), scalar2=None, op0=OP.mult,
                )
                return dfstage

            def emit_dfT(h, dfstage):
                """dfstage rows 32*g4 -> df_cols[:, g4, k] (true df, bf16)."""
                df_cols = sm.tile([128, 4, FCH], BF16, tag=f"dfc{h}", name="dfc")
                for k in range(FCH):
                    tp = mh_ps.tile([128, 128], BF16, tag="mh", name="tp")
                    nc.tensor.transpose(
                        tp, dfstage[:, 128 * k : 128 * (k + 1)], identb
                    )
                    nc.vector.tensor_copy(
                        df_cols[:, :, k : k + 1],
                        tp.rearrange("p (a b) -> p a b", b=32)[:, :, 0:1],
                    )
                return df_cols

            yrow = [None, None]

            def emit_e2_j(h, df_cols, j, fast_q=False):
                """einsum2 j-chunk: yp row 32*g4 = 32*y[g][512j:512j+512];
                evac (descale, bf16) into yrow; after j3, scatter each
                graph row into the y_pm p-major band (4 DMAs)."""
                if j == 0:
                    yrow[h] = sm.tile([128, 2048], BF16, tag=f"yr{h}", name="yr")
                yp = e_ps.tile([128, 512], F32, tag="e", name="yp")
                for k in range(FCH):
                    for g4 in range(4):
                        g = GPH * h + g4
                        nc.tensor.matmul(
                            yp[32 * g4 : 32 * g4 + 1, :],
                            df_cols[:, g4, k : k + 1],
                            sbPT[:, g, k, 512 * j : 512 * (j + 1)],
                            start=(k == 0),
                            stop=(k == FCH - 1),
                            tile_position=(0, 32 * g4),
                        )
                if j in (0, 2):
                    nc.vector.tensor_scalar(
                        out=yrow[h][:, 512 * j : 512 * (j + 1)], in0=yp,
                        scalar1=float(1.0 / PSCALE), scalar2=None, op0=OP.mult,
                    )
                else:
                    nc.scalar.activation(
                        out=yrow[h][:, 512 * j : 512 * (j + 1)], in_=yp,
                        func=ACT.Identity, scale=float(1.0 / PSCALE),
                    )
                if j == 3:
                    for g4 in range(4):
                        if fast_q:
                            eng = nc.scalar
                        else:
                            eng = (nc.sync, nc.gpsimd)[g4 % 2]
                        eng.dma_start(
                            out=y_pm[h][16 * g4 : 16 * g4 + 16, :],
                            in_=yrow[h][32 * g4 : 32 * g4 + 1, :].rearrange(
                                "o (p c) -> o p c", p=16
                            ),
                        )

            def make_tail(s, h):
                """Line search + xs update + next-step prep for half h of
                step s, split into four chained closures (emitted at chosen
                queue positions): p1 = V ratio+min; p2 = PE transpose +
                V per-graph alpha; p2c = PE broadcast (a4T + seg8 matmul);
                p3 = xs update + next-step prep."""
                st_ = {}

                def p1():
                    r = sm.tile([64, 128], F32, tag=f"r{h}", name="r")
                    nc.vector.tensor_mul(r, y_pm[h], xs_inv[h])
                    rmin = sm.tile([64, 1], F32, tag=f"rmin{h}", name="rmin")
                    nc.vector.tensor_reduce(
                        out=rmin, in_=r, axis=AX.X, op=OP.min
                    )
                    st_["rmin"] = rmin

                def p2():
                    rt_ps = mh_ps.tile([1, 64], F32, tag="mh", name="rt_ps")
                    nc.tensor.transpose(rt_ps, st_["rmin"], identf)
                    amin = sm.tile([1, 4], F32, tag=f"am{h}", name="am")
                    nc.vector.tensor_reduce(
                        out=amin,
                        in_=rt_ps.rearrange("o (g b) -> o g b", g=4),
                        axis=AX.X, op=OP.min,
                    )
                    # alpha = 0.995 / max(-rmin, 0.2)  (0.2 <=> step cap 5)
                    nc.vector.tensor_scalar(
                        out=amin, in0=amin, scalar1=float(-1.0 / 0.995),
                        scalar2=float(0.2 / 0.995), op0=OP.mult, op1=OP.max,
                    )
                    nc.vector.reciprocal(amin, amin)
                    st_["amin"] = amin

                def p2c():
                    a4_ps = mh_ps.tile([4, 1], F32, tag="mh", name="a4_ps")
                    nc.tensor.transpose(a4_ps, st_["amin"], identf[0:1, 0:1])
                    a4 = sm.tile([4, 1], F32, tag=f"a4{h}", name="a4")
                    nc.vector.tensor_copy(a4, a4_ps)
                    ac_ps = mh_ps.tile([64, 1], F32, tag="mh", name="ac_ps")
                    nc.tensor.matmul(ac_ps, seg8, a4, start=True, stop=True)
                    acol = sm.tile([64, 1], F32, tag=f"ac{h}", name="ac")
                    nc.vector.tensor_copy(acol, ac_ps)
                    st_["acol"] = acol

                def p3a():
                    nc.vector.scalar_tensor_tensor(
                        out=xs[h], in0=y_pm[h], scalar=st_["acol"], in1=xs[h],
                        op0=OP.mult, op1=OP.add,
                    )
                    if s + 1 >= num_steps:
                        return
                    xbf = sm.tile([64, 128], BF16, tag=f"xbf{h}", name="xbf")
                    nc.vector.tensor_copy(xbf, xs[h])
                    nc.scalar.dma_start(
                        out=rhsx[h][NFEAT : NFEAT + 1, :].rearrange(
                            "o (p c) -> o p c", p=64
                        ),
                        in_=xbf,
                    )
                    nc.gpsimd.dma_start(
                        out=xs_o[s + 1][64 * h : 64 * h + 64, :], in_=xs[h]
                    )

                def p3b():
                    if s + 1 >= num_steps:
                        return
                    nc.vector.reciprocal(out=xs_inv[h], in_=xs[h])
                    t1 = taus[s + 1]
                    rtt = sm.tile([64, 128], F32, tag=f"rtt{h}", name="rtt")
                    nc.vector.tensor_scalar(
                        out=rtt, in0=xs[h], scalar1=float(1.0 / (3.0 * t1)),
                        scalar2=float(1.0 / 3.0), op0=OP.mult, op1=OP.add,
                    )
                    nc.vector.reciprocal(out=rterm[h], in_=rtt)

                return p1, p2, p2c, p3a, p3b

            # =================== the step loop ===================
            # 50%-offset software pipeline.  PE period per half-step:
            #   B:      mlp(h)  (+ other half's dfT + e2 j0/j1 as filler)
            #   pocket: e2 j2/j3 (filler) + this half's d-chain + dT
            #   E:      e1(h)   (+ other half's tail parts injected)
            # The other half's e2/tail "prime" closures carry across the
            # loop body; the last step only needs its mlp blocks.
            dchain_res = {}
            carry = None  # from previous half-1: (dfT1, e2_1 js, tail1 parts)
            for s in range(num_steps):
                last = s == num_steps - 1

                jp = [None] * 4
                if carry is not None:
                    dfT1c, jp, (q1, q2, q2c, q3a, q3b) = carry

                inj = {}
                if carry is not None:
                    inj = {1: [dfT1c], 2: [jp[0]], 3: [jp[1]]}
                emit_mlp(s, 0, injects=inj)

                if last:
                    if carry is not None:
                        jp[2]()
                        jp[3]()
                        q1()
                        q2()
                        q2c()
                        q3a()
                    emit_mlp(s, 1, injects=None)
                    break

                # pocket 0
                e1_inj = {}
                if carry is not None:
                    jp[2]()
                    jp[3]()
                    e1_inj = {3: [q2c], 5: [q3a], 10: [q3b]}
                dchain_res[0] = emit_dchain(0)
                if carry is not None:
                    q1()
                dc0 = emit_dT(0, dchain_res[0])
                if carry is not None:
                    q2()
                dfp0 = emit_e1(0, dc0, injects=e1_inj)
                dfs0 = emit_df_evac(0, dfp0)

                # B3: mlp half 1 with half-0's dfT + e2 j0/j1 as filler
                dfc0_box = {}

                def fill_dfT0():
                    dfc0_box["v"] = emit_dfT(0, dfs0)

                def j0_fill(jj):
                    def f():
                        emit_e2_j(0, dfc0_box["v"], jj)
                    return f

                b3_inj = {1: [fill_dfT0]}
                if s > 0:
                    b3_inj[2] = [j0_fill(0)]
                    b3_inj[3] = [j0_fill(1)]
                emit_mlp(s, 1, injects=b3_inj)
                if s == 0:
                    emit_pt_load(1, None)

                # pocket 1
                p1, p2, p2c, p3a, p3b = make_tail(s, 0)
                if s == 0:
                    emit_e2_j(0, dfc0_box["v"], 0, fast_q=True)
                    emit_e2_j(0, dfc0_box["v"], 1, fast_q=True)
                    emit_e2_j(0, dfc0_box["v"], 2, fast_q=True)
                    emit_e2_j(0, dfc0_box["v"], 3, fast_q=True)
                else:
                    j0_fill(2)()
                    j0_fill(3)()
                dchain_res[1] = emit_dchain(1)
                p1()
                dc1 = emit_dT(1, dchain_res[1])
                p2()
                dfp1 = emit_e1(1, dc1, injects={3: [p2c], 5: [p3a], 10: [p3b]})
                dfs1 = emit_df_evac(1, dfp1)

                dfc1_box = {}

                def fill_dfT1():
                    dfc1_box["v"] = emit_dfT(1, dfs1)

                def j1_fill(jj):
                    def f():
                        emit_e2_j(1, dfc1_box["v"], jj)
                    return f

                carry = (
                    fill_dfT1,
                    [j1_fill(j) for j in range(4)],
                    make_tail(s, 1),
                )

    _split_sync_waits(nc, maxw=1)
    return nc


def _seg_mat():
    seg = np.zeros((64, 64), np.float32)
    for g in range(4):
        seg[16 * g : 16 * g + 16, 16 * g : 16 * g + 16] = 1.0
    seg8 = np.zeros((4, 64), np.float32)
    for g in range(4):
        seg8[g, 16 * g : 16 * g + 16] = 1.0
    return seg, seg8


def _prep_core_inputs(core, proj, x_start, x_solution, node_feat, W1, b1, W2, b2):
    g0 = core * GPC
    n0 = core * NPC
    Pc = proj[g0 : g0 + GPC]  # [8, 2048, 512] f32
    P_f8 = np.ascontiguousarray(
        Pc.reshape(GPC, NCH, 128, F).transpose(2, 0, 1, 3) * PSCALE
    ).astype(F8)
    PT_f8 = np.ascontiguousarray(
        (Pc * PSCALE).transpose(0, 2, 1).reshape(GPC, FCH, 128, NMAX)
        .transpose(2, 0, 1, 3)
    ).astype(F8)
    nfT = np.ascontiguousarray(node_feat[n0 : n0 + NPC].T).astype(BF)
    return {
        "P": P_f8,
        "PT": PT_f8,
        "nfT": nfT,
        "xs0": x_start[n0 : n0 + NPC].reshape(128, 128).astype(np.float32),
        "xsol": x_solution[n0 : n0 + NPC].reshape(128, 128).astype(np.float32),
        "w1": W1.astype(BF),
        "b1": b1.reshape(HID, 1).astype(np.float32),
        "w2": W2.reshape(HID, 1).astype(BF),
        "b2": b2.reshape(1, 1).astype(np.float32),
        "seg": _seg_mat()[0],
        "seg8": _seg_mat()[1],
    }


def _numpy_fallback(x_start, x_solution, node_feat, proj_matrix, W1, b1, W2, b2, batch):
    """General (ragged) reference implementation in numpy, used only if
    vals_batch is not the expected equal-size pattern."""
    nb = proj_matrix.shape[0]
    batch = batch.astype(np.int64)
    counts = np.bincount(batch, minlength=nb)
    offsets = np.cumsum(counts) - counts
    pos = np.arange(batch.shape[0]) - offsets[batch]

    def l1norm(x):
        s = np.zeros(nb, x.dtype)
        np.add.at(s, batch, np.abs(x))
        return x / np.clip(s, 1e-8, None)[batch]

    def to_dense(x):
        dense = np.zeros((nb, NMAX), x.dtype)
        m = pos < NMAX
        dense[batch[m], pos[m]] = x[m]
        return dense

    def line_search(x, dvec):
        neg = dvec < 0
        step = np.where(neg, x / np.where(neg, -dvec, 1.0), STEP_ALPHA)
        a = np.full(nb, np.inf, step.dtype)
        np.minimum.at(a, batch, step)
        return np.minimum(a, STEP_ALPHA)[batch]

    def gnn(x):
        h = np.concatenate([node_feat, x[:, None]], axis=-1)
        h = np.maximum(h @ W1 + b1, 0.0)
        return (h @ W2 + b2)[:, 0]

    tau = 0.01
    xs = x_start.astype(np.float32)
    preds, labels = [], []
    for _ in range(NUM_STEPS):
        pred = gnn(xs)
        preds.append(pred)
        labels.append(l1norm(x_solution - xs))
        p = l1norm(pred)
        direction = p + 3.0 * tau / (xs + tau)
        tau = max(tau * 0.5, 1e-5)
        d_dense = to_dense(direction)
        df = np.einsum("bnf,bn->bf", proj_matrix, d_dense)
        proj_dense = np.einsum("bnf,bf->bn", proj_matrix, df)
        proj_flat = proj_dense[batch, np.minimum(pos, NMAX - 1)]
        proj_flat = np.where(pos < NMAX, proj_flat, 0.0)
        alpha = line_search(xs, proj_flat) * 0.995
        xs = xs + alpha * proj_flat
    return np.stack(preds, 1).astype(np.float32), np.stack(labels, 1).astype(np.float32)


def run_on_hw(inputs_list):
    if "plain" not in _COMPILED:
        _COMPILED["plain"] = build_nc()
    nc = _COMPILED["plain"]
    return run_bass_kernel_spmd(nc, inputs_list, list(range(NCORES))).results


def kernel(x_start, x_solution, node_feat, proj_matrix, W1, b1, W2, b2, vals_batch):
    expected = np.repeat(np.arange(B, dtype=np.int64), NMAX)
    vb = np.asarray(vals_batch)
    if vb.shape != expected.shape or not np.array_equal(
        vb.astype(np.int64), expected
    ):
        return _numpy_fallback(
            np.asarray(x_start, np.float32),
            np.asarray(x_solution, np.float32),
            np.asarray(node_feat, np.float32),
            np.asarray(proj_matrix, np.float32),
            np.asarray(W1, np.float32),
            np.asarray(b1, np.float32),
            np.asarray(W2, np.float32),
            np.asarray(b2, np.float32),
            vb,
        )

    x_start = np.asarray(x_start, np.float32)
    x_solution = np.asarray(x_solution, np.float32)
    node_feat = np.asarray(node_feat, np.float32)
    proj_matrix = np.asarray(proj_matrix, np.float32)
    W1 = np.asarray(W1, np.float32)
    b1 = np.asarray(b1, np.float32)
    W2 = np.asarray(W2, np.float32)
    b2 = np.asarray(b2, np.float32)

    ins = [
        _prep_core_inputs(c, proj_matrix, x_start, x_solution, node_feat, W1, b1, W2, b2)
        for c in range(NCORES)
    ]
    results = run_on_hw(ins)
    preds = np.concatenate(
        [results[c]["preds"].T for c in range(NCORES)], axis=0
    ).astype(np.float32)
    # labels = l1norm(x_solution - xs_s) from the per-step xs snapshots
    xs_all = np.concatenate(
        [results[c]["xs_o"].reshape(NUM_STEPS, NPC) for c in range(NCORES)], axis=1
    )  # [NUM_STEPS, TOTAL]
    diff = x_solution[None, :] - xs_all
    d3 = diff.reshape(NUM_STEPS, B, NMAX)
    sums = np.clip(np.abs(d3).sum(axis=2, keepdims=True), 1e-8, None)
    labels = np.ascontiguousarray(
        (d3 / sums).reshape(NUM_STEPS, B * NMAX).T
    ).astype(np.float32)
    return preds, labels


# revision 38
# speedup vs baseline: 1.0250x; 1.0250x over previous
"""Trainium2 Bass kernel for nn_CycleGNN (8-step projected-direction solver).

Contract: kernel(**inputs) takes the FULL unsharded numpy inputs (keyed as in
setup_inputs()) and returns the full output (preds, labels), each
[131072, 8] float32.  Internally shards the 64 graphs across 8 NeuronCores
(8 graphs per core, graphs never interact -> no collectives), runs a Tile
kernel via run_bass_kernel_spmd, and re-assembles on the host.

Device-side design (per core, 8 graphs, 16384 nodes):
 - The 8 graphs are split into two HALVES (graphs 0-3 / 4-7) with fully
   independent per-half tiles; the two halves' instruction streams are
   interleaved so each half's serial DVE/DMA chains hide under the other
   half's PE bursts (mlp -> d-chain -> einsum1 -> einsum2 -> line search
   is one serial chain per half, but the halves share no data).
 - per-node state is "p-major banded" [64, 128] per half:
   graph q owns partitions [16q, 16q+16); node-within-graph (p%16)*128+c.
 - BOTH P and P^T are SBUF-resident in fp8-e3m4 (scaled by 32), so the
   steady-state loop does no HBM traffic for the projection.
 - einsum1 (df = P^T d) and einsum2 (y = P df) run as 4-way column-tiled
   matvecs (4 graphs concurrent in the PE array via tile_position).
 - line search uses the max-ratio form r = y * (1/xs): the per-element
   divide is replaced by one ACT-LUT reciprocal of xs per step, and the
   per-graph min + broadcast runs on tiny gather/broadcast DMAs (no PE).
"""

import numpy as np
import ml_dtypes

import bass_rust
import concourse.bass as bass
import concourse.tile as tile
from concourse import mybir
from concourse.bass_utils import run_bass_kernel_spmd
from concourse.masks import make_identity

F32 = mybir.dt.float32
BF16 = mybir.dt.bfloat16
FP8 = mybir.dt.float8e3
BF = ml_dtypes.bfloat16
F8 = ml_dtypes.float8_e3m4
PSCALE = 32.0    # P and P^T stored as fp8e3 * PSCALE (absmax ~3.8 < 15.5)

B = 64          # graphs
NMAX = 2048     # nodes per graph (equal-size, sorted vals_batch)
F = 512         # projection basis dim
HID = 128
NFEAT = 64
NUM_STEPS = 8
STEP_ALPHA = 5.0
NCORES = 8
GPC = B // NCORES            # graphs per core = 8
NPC = GPC * NMAX             # nodes per core = 16384
NCH = NMAX // 128            # n-chunks per graph = 16
FCH = F // 128               # f-chunks = 4
GPH = GPC // 2               # graphs per half = 4
NPH = GPH * NMAX             # nodes per half = 8192

AX = mybir.AxisListType
OP = mybir.AluOpType
ACT = mybir.ActivationFunctionType

_COMPILED = {}


def _split_sync_waits(nc, maxw=1):
    """Walrus in this container accepts at most one sync wait per
    instruction; split extra waits into preceding engine-local NoOps."""
    ctr = 0
    for f in nc.m.functions:
        for bb in f.blocks:
            insts = bb.instructions
            out = []
            changed = False
            for ins in insts:
                si = ins.sync_info
                waits = list(si.on_wait) if si is not None else []
                if len(waits) > maxw:
                    reg_waits = [w for w in waits if w.wait_reg is not None]
                    imm_waits = [w for w in waits if w.wait_reg is None]
                    nkeep = max(0, maxw - len(reg_waits))
                    keep = imm_waits[:nkeep]
                    extra = imm_waits[nkeep:]
                    for i in range(0, len(extra), maxw):
                        ctr += 1
                        nop = mybir.InstNoOp(name=f"wsplit-{ctr}", ins=[], outs=[])
                        nop.engine = ins.engine
                        nop.sync_info = bass_rust.SyncInfo(
                            on_wait=extra[i : i + maxw], on_update=[]
                        )
                        out.append(nop)
                    ins.sync_info = bass_rust.SyncInfo(
                        on_wait=reg_waits + keep, on_update=list(si.on_update)
                    )
                    changed = True
                out.append(ins)
            if changed:
                bb.instructions = out
    return ctr


def _tau_schedule():
    taus = []
    tau = 0.01
    for _ in range(NUM_STEPS):
        taus.append(tau)
        tau = max(tau * 0.5, 1e-5)
    return taus


def build_nc(num_steps=NUM_STEPS, debug=False):
    nc = bass.Bass()

    # ---------------- I/O ----------------
    P_d = nc.declare_dram_parameter("P", [128, GPC, NCH, F], FP8, isOutput=False)
    PT_d = nc.declare_dram_parameter("PT", [128, GPC, FCH, NMAX], FP8, isOutput=False)
    nfT_d = nc.declare_dram_parameter("nfT", [NFEAT, NPC], BF16, isOutput=False)
    xs0_d = nc.declare_dram_parameter("xs0", [128, 128], F32, isOutput=False)
    xsol_d = nc.declare_dram_parameter("xsol", [128, 128], F32, isOutput=False)
    w1_d = nc.declare_dram_parameter("w1", [NFEAT + 1, HID], BF16, isOutput=False)
    b1_d = nc.declare_dram_parameter("b1", [HID, 1], F32, isOutput=False)
    w2_d = nc.declare_dram_parameter("w2", [HID, 1], BF16, isOutput=False)
    b2_d = nc.declare_dram_parameter("b2", [1, 1], F32, isOutput=False)
    seg_d = nc.declare_dram_parameter("seg", [64, 64], F32, isOutput=False)
    seg8_d = nc.declare_dram_parameter("seg8", [4, 64], F32, isOutput=False)

    preds_o = nc.declare_dram_parameter("preds", [NUM_STEPS, NPC], F32, isOutput=True)
    # xs snapshot at the START of each step; labels are computed on the host
    xs_o = nc.declare_dram_parameter("xs_o", [NUM_STEPS, 128, 128], F32, isOutput=True)

    taus = _tau_schedule()

    with tile.TileContext(nc) as tc:
        with (
            tc.tile_pool(name="res", bufs=1) as res,            # resident singles
            tc.tile_pool(name="hp", bufs=8) as hp,              # relu'd hidden chunks
            tc.tile_pool(name="st", bufs=1) as st,              # per-half state
            tc.tile_pool(name="sm", bufs=1) as sm,              # small temps
            tc.tile_pool(name="mh_ps", bufs=4, space="PSUM") as mh_ps,
            tc.tile_pool(name="mi_ps", bufs=1, space="PSUM") as mi_ps,
            tc.tile_pool(name="e_ps", bufs=3, space="PSUM") as e_ps,
        ):
            # ---------------- constants / residents ----------------
            identb = res.tile([128, 128], BF16, tag="identb")
            make_identity(nc, identb)

            identf = res.tile([64, 64], F32, tag="identf")
            make_identity(nc, identf)
            seg = res.tile([64, 64], F32, tag="seg")
            nc.sync.dma_start(out=seg, in_=seg_d[:])
            seg8 = res.tile([4, 64], F32, tag="seg8")
            nc.sync.dma_start(out=seg8, in_=seg8_d[:])

            w1 = res.tile([NFEAT + 1, HID], BF16, tag="w1")
            nc.sync.dma_start(out=w1, in_=w1_d[:])
            b1c = res.tile([HID, 1], F32, tag="b1c")
            nc.sync.dma_start(out=b1c, in_=b1_d[:])
            w2 = res.tile([HID, 1], BF16, tag="w2")
            nc.sync.dma_start(out=w2, in_=w2_d[:])
            b2c = res.tile([128, 1], F32, tag="b2c")
            nc.sync.dma_start(
                out=b2c,
                in_=bass.AP(tensor=b2_d, offset=0, ap=[[0, 128], [1, 1]]),
            )

            # per-half mlp moving operand: rows 0..63 node features, row 64 = xs
            rhsx = [res.tile([NFEAT + 1, NPH], BF16, tag=f"rhsx{h}", name="rhsx") for h in (0, 1)]
            for h in (0, 1):
                nc.scalar.dma_start(
                    out=rhsx[h][0:NFEAT, :], in_=nfT_d[:, NPH * h : NPH * (h + 1)]
                )

            # per-half state (all on partitions 0..63)
            xs = [st.tile([64, 128], F32, tag=f"xs{h}", name="xs") for h in (0, 1)]
            xs_inv = [st.tile([64, 128], F32, tag=f"xsi{h}", name="xsi") for h in (0, 1)]
            rterm = [st.tile([64, 128], F32, tag=f"rt{h}", name="rt") for h in (0, 1)]
            pred = [st.tile([64, 128], BF16, tag=f"pred{h}", name="pred") for h in (0, 1)]
            y_pm = [st.tile([64, 128], BF16, tag=f"y{h}", name="ypm") for h in (0, 1)]
            for h in (0, 1):
                nc.gpsimd.dma_start(out=xs[h], in_=xs0_d[64 * h : 64 * h + 64, :])

            # resident P and P^T (fp8 * 32).  Chunked so step-0's small DMAs
            # on the same queues don't wait ~12us behind a monolithic load;
            # PT chunks are emitted inside step 0 (see loop below).
            sbP = res.tile([128, GPC, NCH, F], FP8, tag="sbP")
            sbPT = res.tile([128, GPC, FCH, NMAX], FP8, tag="sbPT")
            # priority order per queue: the first graphs' P and PT come
            # before the later graphs' P (e2_0(0) needs PT g0-3 early)
            for g in range(4):
                eng = (nc.sync, nc.gpsimd)[g % 2]
                eng.dma_start(out=sbP[:, g], in_=P_d[:, g])
            for g in range(4):
                eng = (nc.sync, nc.gpsimd)[g % 2]
                eng.dma_start(out=sbPT[:, g], in_=PT_d[:, g])
            for g in range(4, GPC):
                eng = (nc.sync, nc.gpsimd)[g % 2]
                eng.dma_start(out=sbP[:, g], in_=P_d[:, g])

            def emit_pt_load(h, part):
                # alternate queues per graph so neither DMA queue is
                # blocked for the whole 4MB
                for g4 in (part,) if part is not None else range(GPH):
                    g = GPH * h + g4
                    eng = (nc.sync, nc.gpsimd)[g4 % 2]
                    eng.dma_start(out=sbPT[:, g], in_=PT_d[:, g])

            # ---- step-0 init per half: xs row into rhsx, xs_inv, rterm ----
            for h in (0, 1):
                xbf = sm.tile([64, 128], BF16, tag=f"xbf{h}", name="xbf")
                nc.vector.tensor_copy(xbf, xs[h])
                nc.scalar.dma_start(
                    out=rhsx[h][NFEAT : NFEAT + 1, :].rearrange(
                        "o (p c) -> o p c", p=64
                    ),
                    in_=xbf,
                )
                nc.vector.reciprocal(out=xs_inv[h], in_=xs[h])
                t0 = taus[0]
                rtt = sm.tile([64, 128], F32, tag=f"rtt{h}", name="rtt")
                nc.vector.tensor_scalar(
                    out=rtt, in0=xs[h], scalar1=float(1.0 / (3.0 * t0)),
                    scalar2=float(1.0 / 3.0), op0=OP.mult, op1=OP.add,
                )
                nc.vector.reciprocal(out=rterm[h], in_=rtt)
                nc.gpsimd.dma_start(
                    out=xs_o[0][64 * h : 64 * h + 64, :], in_=xs[h]
                )

            # =================== helper emitters ===================

            def emit_mlp(s, h, injects=None):
                """MLP for half h of step s.  PE order: hg0 hg1 o0 hg2 o1
                hg3 o2 o3 (out-round r consumes hidden group r = chunks
                {4q+r}).  injects: {point: [fn, ...]} emitted at sequence
                points 1=after hg0, 2=after hg1, 3=after o0, 4=after hg2,
                5=after o1, 6=after hg3, 7=after o2, 8=end."""
                injects = injects or {}

                def at(p):
                    for fn in injects.get(p, ()):
                        fn()

                prow = sm.tile([128, 2048], BF16, tag=f"prow{h}", name="prow")
                hq = {}
                tog = [0]

                def hgroup(r):
                    for q in range(4):
                        c = 4 * q + r
                        hps = mh_ps.tile([128, 512], F32, tag="mh", name="hps")
                        nc.tensor.matmul(
                            hps, w1, rhsx[h][:, 512 * c : 512 * (c + 1)],
                            start=True, stop=True,
                        )
                        hpos = hp.tile([128, 512], BF16, tag="h", name="hpos")
                        if tog[0] % 16 in (0, 2, 5, 7, 9, 11, 14):
                            nc.vector.tensor_scalar(
                                out=hpos, in0=hps, scalar1=b1c, scalar2=0.0,
                                op0=OP.add, op1=OP.max,
                            )
                        else:
                            nc.scalar.activation(
                                out=hpos, in_=hps, func=ACT.Relu, bias=b1c
                            )
                        tog[0] += 1
                        hq[c] = hpos

                def oround(r):
                    pp = mi_ps.tile([128, 512], F32, tag="mi", name="pp")
                    for q in range(4):
                        nc.tensor.matmul(
                            pp[32 * q : 32 * q + 1, :],
                            w2, hq[4 * q + r],
                            start=True, stop=True,
                            tile_position=(0, 32 * q),
                        )
                    nc.scalar.activation(
                        out=prow[:, 512 * r : 512 * (r + 1)], in_=pp,
                        func=ACT.Identity, bias=b2c,
                    )
                    # scatter this round's 512-node slice of each graph
                    # into the pred p-major band right away (on V/S queues
                    # for step 0, whose sync/gpsimd queues carry P/PT loads)
                    for q in range(4):
                        if s == 0:
                            eng = nc.scalar
                        elif r == 3:
                            eng = (nc.scalar, nc.sync, nc.gpsimd, nc.scalar)[q]
                        else:
                            eng = (nc.sync, nc.gpsimd)[(r + q) % 2]
                        eng.dma_start(
                            out=pred[h][16 * q + 4 * r : 16 * q + 4 * r + 4, :],
                            in_=prow[
                                32 * q : 32 * q + 1, 512 * r : 512 * (r + 1)
                            ].rearrange("o (p c) -> o p c", p=4),
                        )

                hgroup(0)
                at(1)
                hgroup(1)
                at(2)
                oround(0)
                at(3)
                hgroup(2)
                at(4)
                oround(1)
                at(5)
                hgroup(3)
                at(6)
                oround(2)
                at(7)
                oround(3)
                at(8)
                # preds output straight from row staging (flat node order)
                nc.gpsimd.dma_start(
                    out=preds_o[s, NPH * h : NPH * (h + 1)].rearrange(
                        "(q c) -> q c", q=4
                    ),
                    in_=prow.rearrange("(q o) c -> q o c", q=4)[:, 0:1, :],
                )
                at(9)

            def emit_dchain(h):
                """|pred|_1 per graph -> pscale; d_bf = pred*pscale + rterm.
                Emitted via two parts so the PE op (seg matmul) can sit at a
                chosen PE-queue slot."""
                pp_abs = sm.tile([64, 1], F32, tag=f"pabs{h}", name="pabs")
                junk = sm.tile([64, 128], F32, tag=f"junk{h}", name="junk")
                nc.scalar.activation(
                    out=junk, in_=pred[h], func=ACT.Abs, accum_out=pp_abs
                )
                gs = mi_ps.tile([64, 1], F32, tag="mi", name="gs")
                nc.tensor.matmul(gs, seg, pp_abs, start=True, stop=True)
                pscale = sm.tile([64, 1], F32, tag=f"psc{h}", name="psc")
                nc.vector.reciprocal(pscale, gs)
                d_bf = sm.tile([64, 128], BF16, tag=f"dbf{h}", name="dbf")
                nc.vector.scalar_tensor_tensor(
                    out=d_bf, in0=pred[h], scalar=pscale, in1=rterm[h],
                    op0=OP.mult, op1=OP.add,
                )
                return d_bf

            def emit_dT(h, d_bf):
                """d_bf [64,128] -> d_cols [128,64] via PE transpose.
                d_cols column 16*g4+k = d for (graph g4, node chunk k)."""
                dct = mh_ps.tile([128, 64], BF16, tag="mh", name="dct")
                nc.tensor.transpose(dct, d_bf, identb[0:64, 0:64])
                d_cols = sm.tile([128, 64], BF16, tag=f"dc{h}", name="dc")
                nc.vector.tensor_copy(d_cols, dct)
                return d_cols

            def emit_e1(h, d_cols, injects=None):
                """einsum1: dfp row 32*g4 = 32*df[g]  (4-way col-tiled).
                injects: {k: [fn, ...]} emitted after k-group k."""
                injects = injects or {}
                dfp = e_ps.tile([128, F], F32, tag="e", name="dfp")
                for k in range(NCH):
                    for g4 in range(4):
                        g = GPH * h + g4
                        nc.tensor.matmul(
                            dfp[32 * g4 : 32 * g4 + 1, :],
                            d_cols[:, 16 * g4 + k : 16 * g4 + k + 1],
                            sbP[:, g, k, :],
                            start=(k == 0),
                            stop=(k == NCH - 1),
                            tile_position=(0, 32 * g4),
                        )
                    for fn in injects.get(k, ()):
                        fn()
                return dfp

            def emit_df_evac(h, dfp):
                dfstage = sm.tile([128, F], BF16, tag=f"dfs{h}", name="dfs")
                nc.scalar.activation(
                    out=dfstage, in_=dfp, func=ACT.Identity,
                    scale=float(1.0 / PSCALE),
                )
                return dfstage

            def emit_dfT(h, dfstage):
                """dfstage rows 32*g4 -> df_cols[:, g4, k] (true df, bf16)."""
                df_cols = sm.tile([128, 4, FCH], BF16, tag=f"dfc{h}", name="dfc")
                for k in range(FCH):
                    tp = mh_ps.tile([128, 128], BF16, tag="mh", name="tp")
                    nc.tensor.transpose(
                        tp, dfstage[:, 128 * k : 128 * (k + 1)], identb
                    )
                    nc.vector.tensor_copy(
                        df_cols[:, :, k : k + 1],
                        tp.rearrange("p (a b) -> p a b", b=32)[:, :, 0:1],
                    )
                return df_cols

            yrow = [None, None]

            def emit_e2_j(h, df_cols, j, fast_q=False):
                """einsum2 j-chunk: yp row 32*g4 = 32*y[g][512j:512j+512];
                evac (descale, bf16) into yrow; after j3, scatter each
                graph row into the y_pm p-major band (4 DMAs)."""
                if j == 0:
                    yrow[h] = sm.tile([128, 2048], BF16, tag=f"yr{h}", name="yr")
                yp = e_ps.tile([128, 512], F32, tag="e", name="yp")
                for k in range(FCH):
                    for g4 in range(4):
                        g = GPH * h + g4
                        nc.tensor.matmul(
                            yp[32 * g4 : 32 * g4 + 1, :],
                            df_cols[:, g4, k : k + 1],
                            sbPT[:, g, k, 512 * j : 512 * (j + 1)],
                            start=(k == 0),
                            stop=(k == FCH - 1),
                            tile_position=(0, 32 * g4),
                        )
                if j in (0, 2):
                    nc.vector.tensor_scalar(
                        out=yrow[h][:, 512 * j : 512 * (j + 1)], in0=yp,
                        scalar1=float(1.0 / PSCALE), scalar2=None, op0=OP.mult,
                    )
                else:
                    nc.scalar.activation(
                        out=yrow[h][:, 512 * j : 512 * (j + 1)], in_=yp,
                        func=ACT.Identity, scale=float(1.0 / PSCALE),
                    )
                if j == 3:
                    for g4 in range(4):
                        if fast_q:
                            eng = nc.scalar
                        else:
                            eng = (nc.sync, nc.gpsimd)[g4 % 2]
                        eng.dma_start(
                            out=y_pm[h][16 * g4 : 16 * g4 + 16, :],
                            in_=yrow[h][32 * g4 : 32 * g4 + 1, :].rearrange(
                                "o (p c) -> o p c", p=16
                            ),
                        )

            def make_tail(s, h):
                """Line search + xs update + next-step prep for half h of
                step s, split into four chained closures (emitted at chosen
                queue positions): p1 = V ratio+min; p2 = PE transpose +
                V per-graph alpha; p2c = PE broadcast (a4T + seg8 matmul);
                p3 = xs update + next-step prep."""
                st_ = {}

                def p1():
                    r = sm.tile([64, 128], F32, tag=f"r{h}", name="r")
                    nc.vector.tensor_mul(r, y_pm[h], xs_inv[h])
                    rmin = sm.tile([64, 1], F32, tag=f"rmin{h}", name="rmin")
                    nc.vector.tensor_reduce(
                        out=rmin, in_=r, axis=AX.X, op=OP.min
                    )
                    st_["rmin"] = rmin

                def p2():
                    rt_ps = mh_ps.tile([1, 64], F32, tag="mh", name="rt_ps")
                    nc.tensor.transpose(rt_ps, st_["rmin"], identf)
                    amin = sm.tile([1, 4], F32, tag=f"am{h}", name="am")
                    nc.vector.tensor_reduce(
                        out=amin,
                        in_=rt_ps.rearrange("o (g b) -> o g b", g=4),
                        axis=AX.X, op=OP.min,
                    )
                    # alpha = 0.995 / max(-rmin, 0.2)  (0.2 <=> step cap 5)
                    nc.vector.tensor_scalar(
                        out=amin, in0=amin, scalar1=float(-1.0 / 0.995),
                        scalar2=float(0.2 / 0.995), op0=OP.mult, op1=OP.max,
                    )
                    nc.vector.reciprocal(amin, amin)
                    st_["amin"] = amin

                def p2c():
                    a4_ps = mh_ps.tile([4, 1], F32, tag="mh", name="a4_ps")
                    nc.tensor.transpose(a4_ps, st_["amin"], identf[0:1, 0:1])
                    a4 = sm.tile([4, 1], F32, tag=f"a4{h}", name="a4")
                    nc.vector.tensor_copy(a4, a4_ps)
                    ac_ps = mh_ps.tile([64, 1], F32, tag="mh", name="ac_ps")
                    nc.tensor.matmul(ac_ps, seg8, a4, start=True, stop=True)
                    acol = sm.tile([64, 1], F32, tag=f"ac{h}", name="ac")
                    nc.vector.tensor_copy(acol, ac_ps)
                    st_["acol"] = acol

                def p3a():
                    nc.vector.scalar_tensor_tensor(
                        out=xs[h], in0=y_pm[h], scalar=st_["acol"], in1=xs[h],
                        op0=OP.mult, op1=OP.add,
                    )
                    if s + 1 >= num_steps:
                        return
                    xbf = sm.tile([64, 128], BF16, tag=f"xbf{h}", name="xbf")
                    nc.vector.tensor_copy(xbf, xs[h])
                    nc.scalar.dma_start(
                        out=rhsx[h][NFEAT : NFEAT + 1, :].rearrange(
                            "o (p c) -> o p c", p=64
                        ),
                        in_=xbf,
                    )
                    nc.gpsimd.dma_start(
                        out=xs_o[s + 1][64 * h : 64 * h + 64, :], in_=xs[h]
                    )

                def p3b():
                    if s + 1 >= num_steps:
                        return
                    nc.vector.reciprocal(out=xs_inv[h], in_=xs[h])
                    t1 = taus[s + 1]
                    rtt = sm.tile([64, 128], F32, tag=f"rtt{h}", name="rtt")
                    nc.vector.tensor_scalar(
                        out=rtt, in0=xs[h], scalar1=float(1.0 / (3.0 * t1)),
                        scalar2=float(1.0 / 3.0), op0=OP.mult, op1=OP.add,
                    )
                    nc.vector.reciprocal(out=rterm[h], in_=rtt)

                return p1, p2, p2c, p3a, p3b

            # =================== the step loop ===================
            # 50%-offset software pipeline.  PE period per half-step:
            #   B:      mlp(h)  (+ other half's dfT + e2 j0/j1 as filler)
            #   pocket: e2 j2/j3 (filler) + this half's d-chain + dT
            #   E:      e1(h)   (+ other half's tail parts injected)
            # The other half's e2/tail "prime" closures carry across the
            # loop body; the last step only needs its mlp blocks.
            dchain_res = {}
            carry = None  # from previous half-1: (dfT1, e2_1 js, tail1 parts)
            for s in range(num_steps):
                last = s == num_steps - 1

                jp = [None] * 4
                if carry is not None:
                    dfT1c, jp, (q1, q2, q2c, q3a, q3b) = carry

                inj = {}
                if carry is not None:
                    inj = {1: [dfT1c], 2: [jp[0]], 3: [jp[1]]}
                emit_mlp(s, 0, injects=inj)

                if last:
                    if carry is not None:
                        jp[2]()
                        jp[3]()
                        q1()
                        q2()
                        q2c()
                        q3a()
                    emit_mlp(s, 1, injects=None)
                    break

                # pocket 0
                e1_inj = {}
                if carry is not None:
                    jp[2]()
                    jp[3]()
                    e1_inj = {3: [q2c], 5: [q3a], 10: [q3b]}
                dchain_res[0] = emit_dchain(0)
                if carry is not None:
                    q1()
                dc0 = emit_dT(0, dchain_res[0])
                if carry is not None:
                    q2()
                dfp0 = emit_e1(0, dc0, injects=e1_inj)
                dfs0 = emit_df_evac(0, dfp0)

                # B3: mlp half 1 with half-0's dfT + e2 j0/j1 as filler
                dfc0_box = {}

                def fill_dfT0():
                    dfc0_box["v"] = emit_dfT(0, dfs0)

                def j0_fill(jj):
                    def f():
                        emit_e2_j(0, dfc0_box["v"], jj)
                    return f

                b3_inj = {1: [fill_dfT0]}
                if s > 0:
                    b3_inj[2] = [j0_fill(0)]
                    b3_inj[3] = [j0_fill(1)]
                emit_mlp(s, 1, injects=b3_inj)
                if s == 0:
                    emit_pt_load(1, None)

                # pocket 1
                p1, p2, p2c, p3a, p3b = make_tail(s, 0)
                if s == 0:
                    emit_e2_j(0, dfc0_box["v"], 0, fast_q=True)
                    emit_e2_j(0, dfc0_box["v"], 1, fast_q=True)
                    emit_e2_j(0, dfc0_box["v"], 2, fast_q=True)
                    emit_e2_j(0, dfc0_box["v"], 3, fast_q=True)
                else:
                    j0_fill(2)()
                    j0_fill(3)()
                dchain_res[1] = emit_dchain(1)
                p1()
                dc1 = emit_dT(1, dchain_res[1])
                p2()
                dfp1 = emit_e1(1, dc1, injects={3: [p2c], 5: [p3a], 10: [p3b]})
                dfs1 = emit_df_evac(1, dfp1)

                dfc1_box = {}

                def fill_dfT1():
                    dfc1_box["v"] = emit_dfT(1, dfs1)

                def j1_fill(jj):
                    def f():
                        emit_e2_j(1, dfc1_box["v"], jj)
                    return f

                carry = (
                    fill_dfT1,
                    [j1_fill(j) for j in range(4)],
                    make_tail(s, 1),
                )

    _split_sync_waits(nc, maxw=1)
    return nc


def _seg_mat():
    seg = np.zeros((64, 64), np.float32)
    for g in range(4):
        seg[16 * g : 16 * g + 16, 16 * g : 16 * g + 16] = 1.0
    seg8 = np.zeros((4, 64), np.float32)
    for g in range(4):
        seg8[g, 16 * g : 16 * g + 16] = 1.0
    return seg, seg8


def _prep_core_inputs(core, proj, x_start, x_solution, node_feat, W1, b1, W2, b2):
    g0 = core * GPC
    n0 = core * NPC
    Pc = proj[g0 : g0 + GPC]  # [8, 2048, 512] f32
    P_f8 = np.ascontiguousarray(
        Pc.reshape(GPC, NCH, 128, F).transpose(2, 0, 1, 3) * PSCALE
    ).astype(F8)
    PT_f8 = np.ascontiguousarray(
        (Pc * PSCALE).transpose(0, 2, 1).reshape(GPC, FCH, 128, NMAX)
        .transpose(2, 0, 1, 3)
    ).astype(F8)
    nfT = np.ascontiguousarray(node_feat[n0 : n0 + NPC].T).astype(BF)
    return {
        "P": P_f8,
        "PT": PT_f8,
        "nfT": nfT,
        "xs0": x_start[n0 : n0 + NPC].reshape(128, 128).astype(np.float32),
        "xsol": x_solution[n0 : n0 + NPC].reshape(128, 128).astype(np.float32),
        "w1": W1.astype(BF),
        "b1": b1.reshape(HID, 1).astype(np.float32),
        "w2": W2.reshape(HID, 1).astype(BF),
        "b2": b2.reshape(1, 1).astype(np.float32),
        "seg": _seg_mat()[0],
        "seg8": _seg_mat()[1],
    }


def _numpy_fallback(x_start, x_solution, node_feat, proj_matrix, W1, b1, W2, b2, batch):
    """General (ragged) reference implementation in numpy, used only if
    vals_batch is not the expected equal-size pattern."""
    nb = proj_matrix.shape[0]
    batch = batch.astype(np.int64)
    counts = np.bincount(batch, minlength=nb)
    offsets = np.cumsum(counts) - counts
    pos = np.arange(batch.shape[0]) - offsets[batch]

    def l1norm(x):
        s = np.zeros(nb, x.dtype)
        np.add.at(s, batch, np.abs(x))
        return x / np.clip(s, 1e-8, None)[batch]

    def to_dense(x):
        dense = np.zeros((nb, NMAX), x.dtype)
        m = pos < NMAX
        dense[batch[m], pos[m]] = x[m]
        return dense

    def line_search(x, dvec):
        neg = dvec < 0
        step = np.where(neg, x / np.where(neg, -dvec, 1.0), STEP_ALPHA)
        a = np.full(nb, np.inf, step.dtype)
        np.minimum.at(a, batch, step)
        return np.minimum(a, STEP_ALPHA)[batch]

    def gnn(x):
        h = np.concatenate([node_feat, x[:, None]], axis=-1)
        h = np.maximum(h @ W1 + b1, 0.0)
        return (h @ W2 + b2)[:, 0]

    tau = 0.01
    xs = x_start.astype(np.float32)
    preds, labels = [], []
    for _ in range(NUM_STEPS):
        pred = gnn(xs)
        preds.append(pred)
        labels.append(l1norm(x_solution - xs))
        p = l1norm(pred)
        direction = p + 3.0 * tau / (xs + tau)
        tau = max(tau * 0.5, 1e-5)
        d_dense = to_dense(direction)
        df = np.einsum("bnf,bn->bf", proj_matrix, d_dense)
        proj_dense = np.einsum("bnf,bf->bn", proj_matrix, df)
        proj_flat = proj_dense[batch, np.minimum(pos, NMAX - 1)]
        proj_flat = np.where(pos < NMAX, proj_flat, 0.0)
        alpha = line_search(xs, proj_flat) * 0.995
        xs = xs + alpha * proj_flat
    return np.stack(preds, 1).astype(np.float32), np.stack(labels, 1).astype(np.float32)


def run_on_hw(inputs_list):
    if "plain" not in _COMPILED:
        _COMPILED["plain"] = build_nc()
    nc = _COMPILED["plain"]
    return run_bass_kernel_spmd(nc, inputs_list, list(range(NCORES))).results


def kernel(x_start, x_solution, node_feat, proj_matrix, W1, b1, W2, b2, vals_batch):
    expected = np.repeat(np.arange(B, dtype=np.int64), NMAX)
    vb = np.asarray(vals_batch)
    if vb.shape != expected.shape or not np.array_equal(
        vb.astype(np.int64), expected
    ):
        return _numpy_fallback(
            np.asarray(x_start, np.float32),
            np.asarray(x_solution, np.float32),
            np.asarray(node_feat, np.float32),
            np.asarray(proj_matrix, np.float32),
            np.asarray(W1, np.float32),
            np.asarray(b1, np.float32),
            np.asarray(W2, np.float32),
            np.asarray(b2, np.float32),
            vb,
        )

    x_start = np.asarray(x_start, np.float32)
    x_solution = np.asarray(x_solution, np.float32)
    node_feat = np.asarray(node_feat, np.float32)
    proj_matrix = np.asarray(proj_matrix, np.float32)
    W1 = np.asarray(W1, np.float32)
    b1 = np.asarray(b1, np.float32)
    W2 = np.asarray(W2, np.float32)
    b2 = np.asarray(b2, np.float32)

    ins = [
        _prep_core_inputs(c, proj_matrix, x_start, x_solution, node_feat, W1, b1, W2, b2)
        for c in range(NCORES)
    ]
    results = run_on_hw(ins)
    preds = np.concatenate(
        [results[c]["preds"].T for c in range(NCORES)], axis=0
    ).astype(np.float32)
    # labels = l1norm(x_solution - xs_s) from the per-step xs snapshots
    xs_all = np.concatenate(
        [results[c]["xs_o"].reshape(NUM_STEPS, NPC) for c in range(NCORES)], axis=1
    )  # [NUM_STEPS, TOTAL]
    diff = x_solution[None, :] - xs_all
    d3 = diff.reshape(NUM_STEPS, B, NMAX)
    sums = np.clip(np.abs(d3).sum(axis=2, keepdims=True), 1e-8, None)
    labels = np.ascontiguousarray(
        (d3 / sums).reshape(NUM_STEPS, B * NMAX).T
    ).astype(np.float32)
    return preds, labels


# revision 44
# speedup vs baseline: 1.0423x; 1.0169x over previous
"""Trainium2 Bass kernel for nn_CycleGNN (8-step projected-direction solver).

Contract: kernel(**inputs) takes the FULL unsharded numpy inputs (keyed as in
setup_inputs()) and returns the full output (preds, labels), each
[131072, 8] float32.  Internally shards the 64 graphs across 8 NeuronCores
(8 graphs per core, graphs never interact -> no collectives), runs a Tile
kernel via run_bass_kernel_spmd, and re-assembles on the host.

Device-side design (per core, 8 graphs, 16384 nodes):
 - The 8 graphs are split into two HALVES (graphs 0-3 / 4-7) with fully
   independent per-half tiles; the two halves' instruction streams are
   interleaved so each half's serial DVE/DMA chains hide under the other
   half's PE bursts (mlp -> d-chain -> einsum1 -> einsum2 -> line search
   is one serial chain per half, but the halves share no data).
 - per-node state is "p-major banded" [64, 128] per half:
   graph q owns partitions [16q, 16q+16); node-within-graph (p%16)*128+c.
 - BOTH P and P^T are SBUF-resident in fp8-e3m4 (scaled by 32), so the
   steady-state loop does no HBM traffic for the projection.
 - einsum1 (df = P^T d) and einsum2 (y = P df) run as 4-way column-tiled
   matvecs (4 graphs concurrent in the PE array via tile_position).
 - line search uses the max-ratio form r = y * (1/xs): the per-element
   divide is replaced by one ACT-LUT reciprocal of xs per step, and the
   per-graph min + broadcast runs on tiny gather/broadcast DMAs (no PE).
"""

import numpy as np
import ml_dtypes

import bass_rust
import concourse.bass as bass
import concourse.tile as tile
from concourse import mybir
from concourse.bass_utils import run_bass_kernel_spmd
from concourse.masks import make_identity

F32 = mybir.dt.float32
BF16 = mybir.dt.bfloat16
FP8 = mybir.dt.float8e3
BF = ml_dtypes.bfloat16
F8 = ml_dtypes.float8_e3m4
PSCALE = 32.0    # P and P^T stored as fp8e3 * PSCALE (absmax ~3.8 < 15.5)

B = 64          # graphs
NMAX = 2048     # nodes per graph (equal-size, sorted vals_batch)
F = 512         # projection basis dim
HID = 128
NFEAT = 64
NUM_STEPS = 8
STEP_ALPHA = 5.0
NCORES = 8
GPC = B // NCORES            # graphs per core = 8
NPC = GPC * NMAX             # nodes per core = 16384
NCH = NMAX // 128            # n-chunks per graph = 16
FCH = F // 128               # f-chunks = 4
GPH = GPC // 2               # graphs per half = 4
NPH = GPH * NMAX             # nodes per half = 8192

AX = mybir.AxisListType
OP = mybir.AluOpType
ACT = mybir.ActivationFunctionType

_COMPILED = {}


def _split_sync_waits(nc, maxw=1):
    """Walrus in this container accepts at most one sync wait per
    instruction; split extra waits into preceding engine-local NoOps."""
    ctr = 0
    for f in nc.m.functions:
        for bb in f.blocks:
            insts = bb.instructions
            out = []
            changed = False
            for ins in insts:
                si = ins.sync_info
                waits = list(si.on_wait) if si is not None else []
                if len(waits) > maxw:
                    reg_waits = [w for w in waits if w.wait_reg is not None]
                    imm_waits = [w for w in waits if w.wait_reg is None]
                    nkeep = max(0, maxw - len(reg_waits))
                    keep = imm_waits[:nkeep]
                    extra = imm_waits[nkeep:]
                    for i in range(0, len(extra), maxw):
                        ctr += 1
                        nop = mybir.InstNoOp(name=f"wsplit-{ctr}", ins=[], outs=[])
                        nop.engine = ins.engine
                        nop.sync_info = bass_rust.SyncInfo(
                            on_wait=extra[i : i + maxw], on_update=[]
                        )
                        out.append(nop)
                    ins.sync_info = bass_rust.SyncInfo(
                        on_wait=reg_waits + keep, on_update=list(si.on_update)
                    )
                    changed = True
                out.append(ins)
            if changed:
                bb.instructions = out
    return ctr


def _tau_schedule():
    taus = []
    tau = 0.01
    for _ in range(NUM_STEPS):
        taus.append(tau)
        tau = max(tau * 0.5, 1e-5)
    return taus


def build_nc(num_steps=NUM_STEPS, debug=False):
    nc = bass.Bass()

    # ---------------- I/O ----------------
    P_d = nc.declare_dram_parameter("P", [128, GPC, NCH, F], FP8, isOutput=False)
    PT_d = nc.declare_dram_parameter("PT", [128, GPC, FCH, NMAX], FP8, isOutput=False)
    nfT_d = nc.declare_dram_parameter("nfT", [NFEAT, NPC], BF16, isOutput=False)
    xs0_d = nc.declare_dram_parameter("xs0", [128, 128], F32, isOutput=False)
    xsol_d = nc.declare_dram_parameter("xsol", [128, 128], F32, isOutput=False)
    w1_d = nc.declare_dram_parameter("w1", [NFEAT + 1, HID], BF16, isOutput=False)
    b1_d = nc.declare_dram_parameter("b1", [HID, 1], F32, isOutput=False)
    w2_d = nc.declare_dram_parameter("w2", [HID, 1], BF16, isOutput=False)
    b2_d = nc.declare_dram_parameter("b2", [1, 1], F32, isOutput=False)
    seg_d = nc.declare_dram_parameter("seg", [64, 64], F32, isOutput=False)
    seg8_d = nc.declare_dram_parameter("seg8", [4, 64], F32, isOutput=False)

    preds_o = nc.declare_dram_parameter("preds", [NUM_STEPS, NPC], F32, isOutput=True)
    # xs snapshot at the START of each step; labels are computed on the host
    xs_o = nc.declare_dram_parameter("xs_o", [NUM_STEPS, 128, 128], F32, isOutput=True)

    taus = _tau_schedule()

    with tile.TileContext(nc) as tc:
        with (
            tc.tile_pool(name="res", bufs=1) as res,            # resident singles
            tc.tile_pool(name="hp", bufs=8) as hp,              # relu'd hidden chunks
            tc.tile_pool(name="st", bufs=1) as st,              # per-half state
            tc.tile_pool(name="sm", bufs=1) as sm,              # small temps
            tc.tile_pool(name="mh_ps", bufs=4, space="PSUM") as mh_ps,
            tc.tile_pool(name="mi_ps", bufs=1, space="PSUM") as mi_ps,
            tc.tile_pool(name="e_ps", bufs=3, space="PSUM") as e_ps,
        ):
            # ---------------- constants / residents ----------------
            identb = res.tile([128, 128], BF16, tag="identb")
            make_identity(nc, identb)

            identf = res.tile([64, 64], F32, tag="identf")
            make_identity(nc, identf)
            seg = res.tile([64, 64], F32, tag="seg")
            nc.sync.dma_start(out=seg, in_=seg_d[:])
            seg8 = res.tile([4, 64], F32, tag="seg8")
            nc.sync.dma_start(out=seg8, in_=seg8_d[:])
            third = res.tile([64, 1], F32, tag="third")
            nc.vector.memset(third, float(1.0 / 3.0))

            w1 = res.tile([NFEAT + 1, HID], BF16, tag="w1")
            nc.sync.dma_start(out=w1, in_=w1_d[:])
            b1c = res.tile([HID, 1], F32, tag="b1c")
            nc.sync.dma_start(out=b1c, in_=b1_d[:])
            w2 = res.tile([HID, 1], BF16, tag="w2")
            nc.sync.dma_start(out=w2, in_=w2_d[:])
            b2c = res.tile([128, 1], F32, tag="b2c")
            nc.sync.dma_start(
                out=b2c,
                in_=bass.AP(tensor=b2_d, offset=0, ap=[[0, 128], [1, 1]]),
            )

            # per-half mlp moving operand: rows 0..63 node features, row 64 = xs
            rhsx = [res.tile([NFEAT + 1, NPH], BF16, tag=f"rhsx{h}", name="rhsx") for h in (0, 1)]
            for h in (0, 1):
                nc.scalar.dma_start(
                    out=rhsx[h][0:NFEAT, :], in_=nfT_d[:, NPH * h : NPH * (h + 1)]
                )

            # per-half state (all on partitions 0..63)
            xs = [st.tile([64, 128], F32, tag=f"xs{h}", name="xs") for h in (0, 1)]
            xs_inv = [st.tile([64, 128], F32, tag=f"xsi{h}", name="xsi") for h in (0, 1)]
            rterm = [st.tile([64, 128], F32, tag=f"rt{h}", name="rt") for h in (0, 1)]
            pred = [st.tile([64, 128], BF16, tag=f"pred{h}", name="pred") for h in (0, 1)]
            y_pm = [st.tile([64, 128], BF16, tag=f"y{h}", name="ypm") for h in (0, 1)]
            for h in (0, 1):
                nc.gpsimd.dma_start(out=xs[h], in_=xs0_d[64 * h : 64 * h + 64, :])

            # resident P and P^T (fp8 * 32).  Chunked so step-0's small DMAs
            # on the same queues don't wait ~12us behind a monolithic load;
            # PT chunks are emitted inside step 0 (see loop below).
            sbP = res.tile([128, GPC, NCH, F], FP8, tag="sbP")
            sbPT = res.tile([128, GPC, FCH, NMAX], FP8, tag="sbPT")
            # priority order per queue: the first graphs' P and PT come
            # before the later graphs' P (e2_0(0) needs PT g0-3 early)
            for g in range(4):
                eng = (nc.sync, nc.gpsimd)[g % 2]
                eng.dma_start(out=sbP[:, g], in_=P_d[:, g])
            for g in range(4):
                eng = (nc.sync, nc.gpsimd)[g % 2]
                eng.dma_start(out=sbPT[:, g], in_=PT_d[:, g])
            for g in range(4, GPC):
                eng = (nc.sync, nc.gpsimd)[g % 2]
                eng.dma_start(out=sbP[:, g], in_=P_d[:, g])

            def emit_pt_load(h, part):
                # alternate queues per graph so neither DMA queue is
                # blocked for the whole 4MB
                for g4 in (part,) if part is not None else range(GPH):
                    g = GPH * h + g4
                    eng = (nc.sync, nc.gpsimd)[g4 % 2]
                    eng.dma_start(out=sbPT[:, g], in_=PT_d[:, g])

            # ---- step-0 init per half: xs row into rhsx, xs_inv, rterm ----
            for h in (0, 1):
                xbf = sm.tile([64, 128], BF16, tag=f"xbf{h}", name="xbf")
                nc.vector.tensor_copy(xbf, xs[h])
                nc.scalar.dma_start(
                    out=rhsx[h][NFEAT : NFEAT + 1, :].rearrange(
                        "o (p c) -> o p c", p=64
                    ),
                    in_=xbf,
                )
                nc.vector.reciprocal(out=xs_inv[h], in_=xs[h])
                t0 = taus[0]
                rtt = sm.tile([64, 128], F32, tag=f"rtt{h}", name="rtt")
                nc.vector.tensor_scalar(
                    out=rtt, in0=xs[h], scalar1=float(1.0 / (3.0 * t0)),
                    scalar2=float(1.0 / 3.0), op0=OP.mult, op1=OP.add,
                )
                nc.vector.reciprocal(out=rterm[h], in_=rtt)
                nc.gpsimd.dma_start(
                    out=xs_o[0][64 * h : 64 * h + 64, :], in_=xs[h]
                )

            # =================== helper emitters ===================

            def emit_mlp(s, h, injects=None):
                """MLP for half h of step s.  PE order: hg0 hg1 o0 hg2 o1
                hg3 o2 o3 (out-round r consumes hidden group r = chunks
                {4q+r}).  injects: {point: [fn, ...]} emitted at sequence
                points 1=after hg0, 2=after hg1, 3=after o0, 4=after hg2,
                5=after o1, 6=after hg3, 7=after o2, 8=end."""
                injects = injects or {}

                def at(p):
                    for fn in injects.get(p, ()):
                        fn()

                prow = sm.tile([128, 2048], BF16, tag=f"prow{h}", name="prow")
                hq = {}
                tog = [0]

                def hgroup(r):
                    for q in range(4):
                        c = 4 * q + r
                        hps = mh_ps.tile([128, 512], F32, tag="mh", name="hps")
                        nc.tensor.matmul(
                            hps, w1, rhsx[h][:, 512 * c : 512 * (c + 1)],
                            start=True, stop=True,
                        )
                        hpos = hp.tile([128, 512], BF16, tag="h", name="hpos")
                        if tog[0] % 16 in (0, 2, 5, 7, 9, 11, 14):
                            nc.vector.tensor_scalar(
                                out=hpos, in0=hps, scalar1=b1c, scalar2=0.0,
                                op0=OP.add, op1=OP.max,
                            )
                        else:
                            nc.scalar.activation(
                                out=hpos, in_=hps, func=ACT.Relu, bias=b1c
                            )
                        tog[0] += 1
                        hq[c] = hpos

                def oround(r):
                    pp = mi_ps.tile([128, 512], F32, tag="mi", name="pp")
                    for q in range(4):
                        nc.tensor.matmul(
                            pp[32 * q : 32 * q + 1, :],
                            w2, hq[4 * q + r],
                            start=True, stop=True,
                            tile_position=(0, 32 * q),
                        )
                    nc.scalar.activation(
                        out=prow[:, 512 * r : 512 * (r + 1)], in_=pp,
                        func=ACT.Identity, bias=b2c,
                    )
                    # scatter this round's 512-node slice of each graph
                    # into the pred p-major band right away (on V/S queues
                    # for step 0, whose sync/gpsimd queues carry P/PT loads)
                    for q in range(4):
                        if s == 0:
                            eng = nc.scalar
                        elif r == 3:
                            eng = (nc.scalar, nc.sync, nc.gpsimd, nc.scalar)[q]
                        else:
                            eng = (nc.sync, nc.gpsimd)[(r + q) % 2]
                        eng.dma_start(
                            out=pred[h][16 * q + 4 * r : 16 * q + 4 * r + 4, :],
                            in_=prow[
                                32 * q : 32 * q + 1, 512 * r : 512 * (r + 1)
                            ].rearrange("o (p c) -> o p c", p=4),
                        )

                hgroup(0)
                at(1)
                hgroup(1)
                at(2)
                oround(0)
                at(3)
                hgroup(2)
                at(4)
                oround(1)
                at(5)
                hgroup(3)
                at(6)
                oround(2)
                at(7)
                oround(3)
                at(8)
                # preds output straight from row staging (flat node order)
                nc.gpsimd.dma_start(
                    out=preds_o[s, NPH * h : NPH * (h + 1)].rearrange(
                        "(q c) -> q c", q=4
                    ),
                    in_=prow.rearrange("(q o) c -> q o c", q=4)[:, 0:1, :],
                )
                at(9)

            def emit_dchain(h):
                """|pred|_1 per graph -> pscale; d_bf = pred*pscale + rterm.
                Emitted via two parts so the PE op (seg matmul) can sit at a
                chosen PE-queue slot."""
                pp_abs = sm.tile([64, 1], F32, tag=f"pabs{h}", name="pabs")
                junk = sm.tile([64, 128], F32, tag=f"junk{h}", name="junk")
                nc.scalar.activation(
                    out=junk, in_=pred[h], func=ACT.Abs, accum_out=pp_abs
                )
                gs = mi_ps.tile([64, 1], F32, tag="mi", name="gs")
                nc.tensor.matmul(gs, seg, pp_abs, start=True, stop=True)
                pscale = sm.tile([64, 1], F32, tag=f"psc{h}", name="psc")
                nc.vector.reciprocal(pscale, gs)
                d_bf = sm.tile([64, 128], BF16, tag=f"dbf{h}", name="dbf")
                nc.vector.scalar_tensor_tensor(
                    out=d_bf, in0=pred[h], scalar=pscale, in1=rterm[h],
                    op0=OP.mult, op1=OP.add,
                )
                return d_bf

            def emit_dT(h, d_bf):
                """d_bf [64,128] -> d_cols [128,64] via PE transpose.
                d_cols column 16*g4+k = d for (graph g4, node chunk k)."""
                dct = mh_ps.tile([128, 64], BF16, tag="mh", name="dct")
                nc.tensor.transpose(dct, d_bf, identb[0:64, 0:64])
                d_cols = sm.tile([128, 64], BF16, tag=f"dc{h}", name="dc")
                nc.vector.tensor_copy(d_cols, dct)
                return d_cols

            def emit_e1(h, d_cols, injects=None):
                """einsum1: dfp row 32*g4 = 32*df[g]  (4-way col-tiled).
                injects: {k: [fn, ...]} emitted after k-group k."""
                injects = injects or {}
                dfp = e_ps.tile([128, F], F32, tag="e", name="dfp")
                for k in range(NCH):
                    for g4 in range(4):
                        g = GPH * h + g4
                        nc.tensor.matmul(
                            dfp[32 * g4 : 32 * g4 + 1, :],
                            d_cols[:, 16 * g4 + k : 16 * g4 + k + 1],
                            sbP[:, g, k, :],
                            start=(k == 0),
                            stop=(k == NCH - 1),
                            tile_position=(0, 32 * g4),
                        )
                    for fn in injects.get(k, ()):
                        fn()
                return dfp

            def emit_df_evac(h, dfp):
                dfstage = sm.tile([128, F], BF16, tag=f"dfs{h}", name="dfs")
                nc.scalar.activation(
                    out=dfstage, in_=dfp, func=ACT.Identity,
                    scale=float(1.0 / PSCALE),
                )
                return dfstage

            def emit_dfT(h, dfstage):
                """dfstage rows 32*g4 -> df_cols[:, g4, k] (true df, bf16)."""
                df_cols = sm.tile([128, 4, FCH], BF16, tag=f"dfc{h}", name="dfc")
                for k in range(FCH):
                    tp = mh_ps.tile([128, 128], BF16, tag="mh", name="tp")
                    nc.tensor.transpose(
                        tp, dfstage[:, 128 * k : 128 * (k + 1)], identb
                    )
                    nc.vector.tensor_copy(
                        df_cols[:, :, k : k + 1],
                        tp.rearrange("p (a b) -> p a b", b=32)[:, :, 0:1],
                    )
                return df_cols

            yrow = [None, None]

            def emit_e2_j(h, df_cols, j, fast_q=False):
                """einsum2 j-chunk: yp row 32*g4 = 32*y[g][512j:512j+512];
                evac (descale, bf16) into yrow; after j3, scatter each
                graph row into the y_pm p-major band (4 DMAs)."""
                if j == 0:
                    yrow[h] = sm.tile([128, 2048], BF16, tag=f"yr{h}", name="yr")
                yp = e_ps.tile([128, 512], F32, tag="e", name="yp")
                for k in range(FCH):
                    for g4 in range(4):
                        g = GPH * h + g4
                        nc.tensor.matmul(
                            yp[32 * g4 : 32 * g4 + 1, :],
                            df_cols[:, g4, k : k + 1],
                            sbPT[:, g, k, 512 * j : 512 * (j + 1)],
                            start=(k == 0),
                            stop=(k == FCH - 1),
                            tile_position=(0, 32 * g4),
                        )
                if j in (0, 2):
                    nc.vector.tensor_scalar(
                        out=yrow[h][:, 512 * j : 512 * (j + 1)], in0=yp,
                        scalar1=float(1.0 / PSCALE), scalar2=None, op0=OP.mult,
                    )
                else:
                    nc.scalar.activation(
                        out=yrow[h][:, 512 * j : 512 * (j + 1)], in_=yp,
                        func=ACT.Identity, scale=float(1.0 / PSCALE),
                    )
                if j == 3:
                    for g4 in range(4):
                        if fast_q:
                            eng = nc.scalar
                        else:
                            eng = (nc.sync, nc.gpsimd)[g4 % 2]
                        eng.dma_start(
                            out=y_pm[h][16 * g4 : 16 * g4 + 16, :],
                            in_=yrow[h][32 * g4 : 32 * g4 + 1, :].rearrange(
                                "o (p c) -> o p c", p=16
                            ),
                        )

            def make_tail(s, h):
                """Line search + xs update + next-step prep for half h of
                step s, split into four chained closures (emitted at chosen
                queue positions): p1 = V ratio+min; p2 = PE transpose +
                V per-graph alpha; p2c = PE broadcast (a4T + seg8 matmul);
                p3 = xs update + next-step prep."""
                st_ = {}

                def p1():
                    r = sm.tile([64, 128], F32, tag=f"r{h}", name="r")
                    nc.vector.tensor_mul(r, y_pm[h], xs_inv[h])
                    rmin = sm.tile([64, 1], F32, tag=f"rmin{h}", name="rmin")
                    nc.vector.tensor_reduce(
                        out=rmin, in_=r, axis=AX.X, op=OP.min
                    )
                    st_["rmin"] = rmin

                def p2():
                    rt_ps = mh_ps.tile([1, 64], F32, tag="mh", name="rt_ps")
                    nc.tensor.transpose(rt_ps, st_["rmin"], identf)
                    amin = sm.tile([1, 4], F32, tag=f"am{h}", name="am")
                    nc.vector.tensor_reduce(
                        out=amin,
                        in_=rt_ps.rearrange("o (g b) -> o g b", g=4),
                        axis=AX.X, op=OP.min,
                    )
                    # alpha = 0.995 / max(-rmin, 0.2)  (0.2 <=> step cap 5)
                    nc.vector.tensor_scalar(
                        out=amin, in0=amin, scalar1=float(-1.0 / 0.995),
                        scalar2=float(0.2 / 0.995), op0=OP.mult, op1=OP.max,
                    )
                    nc.vector.reciprocal(amin, amin)
                    st_["amin"] = amin

                def p2c():
                    a4_ps = mh_ps.tile([4, 1], F32, tag="mh", name="a4_ps")
                    nc.tensor.transpose(a4_ps, st_["amin"], identf[0:1, 0:1])
                    a4 = sm.tile([4, 1], F32, tag=f"a4{h}", name="a4")
                    nc.vector.tensor_copy(a4, a4_ps)
                    ac_ps = mh_ps.tile([64, 1], F32, tag="mh", name="ac_ps")
                    nc.tensor.matmul(ac_ps, seg8, a4, start=True, stop=True)
                    acol = sm.tile([64, 1], F32, tag=f"ac{h}", name="ac")
                    nc.vector.tensor_copy(acol, ac_ps)
                    st_["acol"] = acol

                def p3a():
                    nc.vector.scalar_tensor_tensor(
                        out=xs[h], in0=y_pm[h], scalar=st_["acol"], in1=xs[h],
                        op0=OP.mult, op1=OP.add,
                    )
                    if s + 1 >= num_steps:
                        return
                    xbf = sm.tile([64, 128], BF16, tag=f"xbf{h}", name="xbf")
                    nc.vector.tensor_copy(xbf, xs[h])
                    nc.scalar.dma_start(
                        out=rhsx[h][NFEAT : NFEAT + 1, :].rearrange(
                            "o (p c) -> o p c", p=64
                        ),
                        in_=xbf,
                    )
                    nc.gpsimd.dma_start(
                        out=xs_o[s + 1][64 * h : 64 * h + 64, :], in_=xs[h]
                    )

                def p3b():
                    if s + 1 >= num_steps:
                        return
                    nc.vector.reciprocal(out=xs_inv[h], in_=xs[h])
                    t1 = taus[s + 1]
                    rtt = sm.tile([64, 128], F32, tag=f"rtt{h}", name="rtt")
                    nc.scalar.activation(
                        out=rtt, in_=xs[h], func=ACT.Identity,
                        scale=float(1.0 / (3.0 * t1)), bias=third,
                    )
                    nc.vector.reciprocal(out=rterm[h], in_=rtt)

                return p1, p2, p2c, p3a, p3b

            # =================== the step loop ===================
            # 50%-offset software pipeline.  PE period per half-step:
            #   B:      mlp(h)  (+ other half's dfT + e2 j0/j1 as filler)
            #   pocket: e2 j2/j3 (filler) + this half's d-chain + dT
            #   E:      e1(h)   (+ other half's tail parts injected)
            # The other half's e2/tail "prime" closures carry across the
            # loop body; the last step only needs its mlp blocks.
            dchain_res = {}
            carry = None  # from previous half-1: (dfT1, e2_1 js, tail1 parts)
            for s in range(num_steps):
                last = s == num_steps - 1

                jp = [None] * 4
                if carry is not None:
                    dfT1c, jp, (q1, q2, q2c, q3a, q3b) = carry

                inj = {}
                if carry is not None:
                    inj = {1: [dfT1c], 2: [jp[0]], 3: [jp[1]]}
                emit_mlp(s, 0, injects=inj)

                if last:
                    if carry is not None:
                        jp[2]()
                        jp[3]()
                        q1()
                        q2()
                        q2c()
                        q3a()
                    emit_mlp(s, 1, injects=None)
                    break

                # pocket 0
                e1_inj = {}
                if carry is not None:
                    jp[2]()
                    jp[3]()
                    e1_inj = {3: [q2c], 5: [q3a], 10: [q3b]}
                dchain_res[0] = emit_dchain(0)
                if carry is not None:
                    q1()
                dc0 = emit_dT(0, dchain_res[0])
                if carry is not None:
                    q2()
                dfp0 = emit_e1(0, dc0, injects=e1_inj)
                dfs0 = emit_df_evac(0, dfp0)

                # B3: mlp half 1 with half-0's dfT + e2 j0/j1 as filler
                dfc0_box = {}

                def fill_dfT0():
                    dfc0_box["v"] = emit_dfT(0, dfs0)

                def j0_fill(jj):
                    def f():
                        emit_e2_j(0, dfc0_box["v"], jj)
                    return f

                b3_inj = {1: [fill_dfT0]}
                if s > 0:
                    b3_inj[2] = [j0_fill(0)]
                    b3_inj[3] = [j0_fill(1)]
                emit_mlp(s, 1, injects=b3_inj)
                if s == 0:
                    emit_pt_load(1, None)

                # pocket 1
                p1, p2, p2c, p3a, p3b = make_tail(s, 0)
                if s == 0:
                    emit_e2_j(0, dfc0_box["v"], 0, fast_q=True)
                    emit_e2_j(0, dfc0_box["v"], 1, fast_q=True)
                    emit_e2_j(0, dfc0_box["v"], 2, fast_q=True)
                    emit_e2_j(0, dfc0_box["v"], 3, fast_q=True)
                else:
                    j0_fill(2)()
                    j0_fill(3)()
                dchain_res[1] = emit_dchain(1)
                p1()
                dc1 = emit_dT(1, dchain_res[1])
                p2()
                dfp1 = emit_e1(1, dc1, injects={3: [p2c], 5: [p3a], 10: [p3b]})
                dfs1 = emit_df_evac(1, dfp1)

                dfc1_box = {}

                def fill_dfT1():
                    dfc1_box["v"] = emit_dfT(1, dfs1)

                def j1_fill(jj):
                    def f():
                        emit_e2_j(1, dfc1_box["v"], jj)
                    return f

                carry = (
                    fill_dfT1,
                    [j1_fill(j) for j in range(4)],
                    make_tail(s, 1),
                )

    _split_sync_waits(nc, maxw=1)
    return nc


def _seg_mat():
    seg = np.zeros((64, 64), np.float32)
    for g in range(4):
        seg[16 * g : 16 * g + 16, 16 * g : 16 * g + 16] = 1.0
    seg8 = np.zeros((4, 64), np.float32)
    for g in range(4):
        seg8[g, 16 * g : 16 * g + 16] = 1.0
    return seg, seg8


def _prep_core_inputs(core, proj, x_start, x_solution, node_feat, W1, b1, W2, b2):
    g0 = core * GPC
    n0 = core * NPC
    Pc = proj[g0 : g0 + GPC]  # [8, 2048, 512] f32
    P_f8 = np.ascontiguousarray(
        Pc.reshape(GPC, NCH, 128, F).transpose(2, 0, 1, 3) * PSCALE
    ).astype(F8)
    PT_f8 = np.ascontiguousarray(
        (Pc * PSCALE).transpose(0, 2, 1).reshape(GPC, FCH, 128, NMAX)
        .transpose(2, 0, 1, 3)
    ).astype(F8)
    nfT = np.ascontiguousarray(node_feat[n0 : n0 + NPC].T).astype(BF)
    return {
        "P": P_f8,
        "PT": PT_f8,
        "nfT": nfT,
        "xs0": x_start[n0 : n0 + NPC].reshape(128, 128).astype(np.float32),
        "xsol": x_solution[n0 : n0 + NPC].reshape(128, 128).astype(np.float32),
        "w1": W1.astype(BF),
        "b1": b1.reshape(HID, 1).astype(np.float32),
        "w2": W2.reshape(HID, 1).astype(BF),
        "b2": b2.reshape(1, 1).astype(np.float32),
        "seg": _seg_mat()[0],
        "seg8": _seg_mat()[1],
    }


def _numpy_fallback(x_start, x_solution, node_feat, proj_matrix, W1, b1, W2, b2, batch):
    """General (ragged) reference implementation in numpy, used only if
    vals_batch is not the expected equal-size pattern."""
    nb = proj_matrix.shape[0]
    batch = batch.astype(np.int64)
    counts = np.bincount(batch, minlength=nb)
    offsets = np.cumsum(counts) - counts
    pos = np.arange(batch.shape[0]) - offsets[batch]

    def l1norm(x):
        s = np.zeros(nb, x.dtype)
        np.add.at(s, batch, np.abs(x))
        return x / np.clip(s, 1e-8, None)[batch]

    def to_dense(x):
        dense = np.zeros((nb, NMAX), x.dtype)
        m = pos < NMAX
        dense[batch[m], pos[m]] = x[m]
        return dense

    def line_search(x, dvec):
        neg = dvec < 0
        step = np.where(neg, x / np.where(neg, -dvec, 1.0), STEP_ALPHA)
        a = np.full(nb, np.inf, step.dtype)
        np.minimum.at(a, batch, step)
        return np.minimum(a, STEP_ALPHA)[batch]

    def gnn(x):
        h = np.concatenate([node_feat, x[:, None]], axis=-1)
        h = np.maximum(h @ W1 + b1, 0.0)
        return (h @ W2 + b2)[:, 0]

    tau = 0.01
    xs = x_start.astype(np.float32)
    preds, labels = [], []
    for _ in range(NUM_STEPS):
        pred = gnn(xs)
        preds.append(pred)
        labels.append(l1norm(x_solution - xs))
        p = l1norm(pred)
        direction = p + 3.0 * tau / (xs + tau)
        tau = max(tau * 0.5, 1e-5)
        d_dense = to_dense(direction)
        df = np.einsum("bnf,bn->bf", proj_matrix, d_dense)
        proj_dense = np.einsum("bnf,bf->bn", proj_matrix, df)
        proj_flat = proj_dense[batch, np.minimum(pos, NMAX - 1)]
        proj_flat = np.where(pos < NMAX, proj_flat, 0.0)
        alpha = line_search(xs, proj_flat) * 0.995
        xs = xs + alpha * proj_flat
    return np.stack(preds, 1).astype(np.float32), np.stack(labels, 1).astype(np.float32)


def run_on_hw(inputs_list):
    if "plain" not in _COMPILED:
        _COMPILED["plain"] = build_nc()
    nc = _COMPILED["plain"]
    return run_bass_kernel_spmd(nc, inputs_list, list(range(NCORES))).results


def kernel(x_start, x_solution, node_feat, proj_matrix, W1, b1, W2, b2, vals_batch):
    expected = np.repeat(np.arange(B, dtype=np.int64), NMAX)
    vb = np.asarray(vals_batch)
    if vb.shape != expected.shape or not np.array_equal(
        vb.astype(np.int64), expected
    ):
        return _numpy_fallback(
            np.asarray(x_start, np.float32),
            np.asarray(x_solution, np.float32),
            np.asarray(node_feat, np.float32),
            np.asarray(proj_matrix, np.float32),
            np.asarray(W1, np.float32),
            np.asarray(b1, np.float32),
            np.asarray(W2, np.float32),
            np.asarray(b2, np.float32),
            vb,
        )

    x_start = np.asarray(x_start, np.float32)
    x_solution = np.asarray(x_solution, np.float32)
    node_feat = np.asarray(node_feat, np.float32)
    proj_matrix = np.asarray(proj_matrix, np.float32)
    W1 = np.asarray(W1, np.float32)
    b1 = np.asarray(b1, np.float32)
    W2 = np.asarray(W2, np.float32)
    b2 = np.asarray(b2, np.float32)

    ins = [
        _prep_core_inputs(c, proj_matrix, x_start, x_solution, node_feat, W1, b1, W2, b2)
        for c in range(NCORES)
    ]
    results = run_on_hw(ins)
    preds = np.concatenate(
        [results[c]["preds"].T for c in range(NCORES)], axis=0
    ).astype(np.float32)
    # labels = l1norm(x_solution - xs_s) from the per-step xs snapshots
    xs_all = np.concatenate(
        [results[c]["xs_o"].reshape(NUM_STEPS, NPC) for c in range(NCORES)], axis=1
    )  # [NUM_STEPS, TOTAL]
    diff = x_solution[None, :] - xs_all
    d3 = diff.reshape(NUM_STEPS, B, NMAX)
    sums = np.clip(np.abs(d3).sum(axis=2, keepdims=True), 1e-8, None)
    labels = np.ascontiguousarray(
        (d3 / sums).reshape(NUM_STEPS, B * NMAX).T
    ).astype(np.float32)
    return preds, labels
